# revision 59
# baseline (speedup 1.0000x reference)
"""GPT2Attention Trainium2 Bass kernel.

Problem: B=2, S=2048, E=1024, H=16 heads, D=64.
  qkv = x @ c_attn_w + c_attn_b; causal softmax attention; out @ c_proj_w + c_proj_b.

Sharding: 8 cores = 2 (batch) x 4 (head-groups of 4 heads).  Each core computes
its batch's attention for its 4 heads plus the partial c_proj contribution
(rows of c_proj_w belonging to its heads).  Host sums the 4 partials per batch
and adds the bias terms (v-bias folds through attention: attn rows sum to 1).

Device pipeline, software-pipelined over 512-query chunks so the tile
scheduler can overlap the PE-heavy projections with the ACT-heavy softmax:

  A(c): QKV projection for query chunk c.  qT/kT per head-pair [128, S]
        (partitions = 2 heads x 64 dims); v natural [tokens, 4 heads, 65]
        with a ones column that accumulates the softmax denominator.
  B(c): per head: scoresT tiles land in [128 keys, 2x512 q] PSUM pairs so a
        single ACT exp covers two key tiles; moving operands are trimmed to
        the causal boundary, and the two diagonal-block pairs pack as a
        (3,0)/(2,1) "butterfly" to minimise dead exp work.  Boundary
        regions are zeroed by an in-place bf16 multiply with a triangular
        predicate on DVE.  attn@v runs transposed -- po[128 q, 65]
        accumulates over key tiles at 65 rows/matmul instead of 512 --
        and one broadcast reciprocal-multiply normalizes a whole head.
  T(c): c2 [tok, dims] -> cT [dims, tok] via DMA-transpose mid-stream
        (idle DMA hardware); via PE-transpose for the latency-critical
        last chunk.
  C(c): partial c_proj from cT, evicted to SBUF (DVE mid-stream, the
        then-idle ACT for the last chunk) and shipped as bf16.

The emission order software-pipelines chunks: chunk c's scores/exps are
issued before chunk c-1's attn@v + normalize + c_proj, so ACT always has
exp work queued while PE turns through reductions.  A 10-matmul warmup
keeps the PE p-state ramp hot through the initial DMA window.
"""

from contextlib import ExitStack

import numpy as np
import ml_dtypes

import bass_rust
import concourse.bass as bass
import concourse.tile as tile
from concourse import mybir
from concourse import bass_utils


def _patched_drain_and_barrier(self, tick_clock, wait_clock):
    # The stock walrus in this container rejects instructions carrying more
    # than one sync wait ("Too many sync wait commands" on the kernel-tail
    # Drain).  Spread the final waits across single-wait NOPs instead.
    nc = self.nc
    probe = nc.sync.nop()
    wait_clock.add_sem_waits(
        probe.ins, bass_rust.ScopedClock({None: tick_clock.global_clock}))
    si = probe.ins.sync_info
    waits = list(si.on_wait) if si is not None else []
    if len(waits) > 1:
        probe.ins.sync_info = mybir.SyncInfo(
            on_wait=waits[:1], on_update=list(si.on_update))
        for w in waits[1:]:
            n = nc.sync.nop()
            n.ins.sync_info = mybir.SyncInfo(on_wait=[w], on_update=[])
    nc.sync.drain()
    nc.all_engine_barrier()
    assert self.sems is not None
    popped = nc._tile_sem_poison_stack.pop()
    assert popped is self._sem_poison
    nc.clear_and_free_semaphores(list(self.sems.allocated().values()))
    nc.all_engine_barrier()


tile.TileContext._drain_and_barrier = _patched_drain_and_barrier

_split_ctr = [0]


def _split_sync_waits(nc):
    """Stock walrus allows one sync wait per instruction; hoist extras onto
    single-wait NOPs inserted just before, on the same (in-order) engine."""
    for fn in nc.m.functions:
        for bb in fn.blocks:
            insts = bb.instructions
            out = []
            changed = False
            for inst in insts:
                si = getattr(inst, "sync_info", None)
                waits = list(si.on_wait) if si is not None else []
                if len(waits) > 1:
                    changed = True
                    for w in waits[:-1]:
                        _split_ctr[0] += 1
                        nop = bass_rust.InstNoOp(
                            name=f"I-syncsplit-{_split_ctr[0]}",
                            engine=inst.engine)
                        nop.sync_info = mybir.SyncInfo(on_wait=[w], on_update=[])
                        out.append(nop)
                    inst.sync_info = mybir.SyncInfo(
                        on_wait=[waits[-1]], on_update=list(si.on_update))
                out.append(inst)
            if changed:
                bb.instructions = out

B, S, E, H, D = 2, 2048, 1024, 16, 64
NCORES = 8
HG = 4                # head-group cores per batch
LH = H // HG          # 4 local heads per core
LC = LH * D           # 256 local c_proj rows
NPAIR = LH // 2       # 2 head-pairs per core
P = 128
KT = E // P           # 8 contraction tiles for the projections
QCHUNK = 512
NQC = S // QCHUNK     # 4 query chunks
NKT = S // P          # 16 key tiles
NTT = S // P          # 16 token tiles

FP = mybir.dt.float32
BF = mybir.dt.bfloat16
EXP = mybir.ActivationFunctionType.Exp
COPY = mybir.ActivationFunctionType.Copy


def _build_module():
    nc = bass.Bass("TRN2", target_bir_lowering=False, debug=False,
                   num_devices=NCORES)
    xT = nc.dram_tensor("xT", [E, S], BF, kind="ExternalInput").ap()
    wq = nc.dram_tensor("wq", [P, NPAIR * KT * P], BF, kind="ExternalInput").ap()
    wk = nc.dram_tensor("wk", [P, NPAIR * KT * P], BF, kind="ExternalInput").ap()
    wv = nc.dram_tensor("wv", [E, LC], BF, kind="ExternalInput").ap()
    w2 = nc.dram_tensor("w2", [LC, E], BF, kind="ExternalInput").ap()
    tri = nc.dram_tensor("tri", [P, 3 * P], BF, kind="ExternalInput").ap()
    bqk = nc.dram_tensor("bqk", [P, 2 * NPAIR], FP, kind="ExternalInput").ap()
    ident = nc.dram_tensor("ident", [P, P], BF, kind="ExternalInput").ap()
    y = nc.dram_tensor("y", [S, E], BF, kind="ExternalOutput").ap()

    with tile.TileContext(nc) as tc:
        _body(tc, xT, wq, wk, wv, w2, tri, bqk, ident, y)
    _split_sync_waits(nc)
    return nc


def _body(tc, xT, wq, wk, wv, w2, tri, bqk, ident, y):
    nc = tc.nc
    ex = ExitStack()
    with ex:
        persist = ex.enter_context(tc.tile_pool(name="persist", bufs=1))

        # ---- persistent tiles ----
        qT2 = [persist.tile([P, S], BF, name=f"qT2_{p}") for p in range(NPAIR)]
        kT2 = [persist.tile([P, S], BF, name=f"kT2_{p}") for p in range(NPAIR)]
        # v natural layout: [token-partitions, ttile, head, dim(+denom col)]
        vall = persist.tile([P, NTT, LH, D + 1], BF, name="vall")
        cT = [persist.tile([P, S], BF, name=f"cT_{p}") for p in range(NPAIR)]
        xt_sb = persist.tile([P, KT, S], BF, name="xt_sb")
        wq_sb = persist.tile([P, NPAIR, KT, P], BF, name="wq_sb")
        wk_sb = persist.tile([P, NPAIR, KT, P], BF, name="wk_sb")
        wv_sb = persist.tile([P, KT, LC], BF, name="wv_sb")
        w2_sb = persist.tile([P, 2, E], BF, name="w2_sb")
        # masking predicates for the causal boundary: cols 0:128 = (j < k),
        # 128:384 = (j < 128+k)
        tri_sb = persist.tile([P, 3 * P], BF, name="tri_sb")
        bqk_sb = persist.tile([P, 2 * NPAIR], FP, name="bqk_sb")
        ident_sb = persist.tile([P, P], BF, name="ident_sb")

        nc.vector.memset(vall[:, :, :, D:D + 1], 1.0)

        # ---- input DMAs (ordered so chunk-0 work can start early) ----
        xTr = xT.rearrange("(k p) s -> k p s", p=P)
        KP = KT * P
        nc.sync.dma_start(out=wq_sb[:, 0], in_=wq[:, 0:KP].rearrange(
            "p (k j) -> p k j", k=KT))
        nc.sync.dma_start(out=wk_sb[:, 0], in_=wk[:, 0:KP].rearrange(
            "p (k j) -> p k j", k=KT))
        nc.sync.dma_start(out=xt_sb[:, 0:4, 0:QCHUNK],
                          in_=xTr[0:4, :, 0:QCHUNK].rearrange("k p s -> p k s"))
        nc.sync.dma_start(out=xt_sb[:, 4:8, 0:QCHUNK],
                          in_=xTr[4:8, :, 0:QCHUNK].rearrange("k p s -> p k s"))
        nc.sync.dma_start(out=wq_sb[:, 1], in_=wq[:, KP:2 * KP].rearrange(
            "p (k j) -> p k j", k=KT))
        nc.sync.dma_start(out=wk_sb[:, 1], in_=wk[:, KP:2 * KP].rearrange(
            "p (k j) -> p k j", k=KT))
        nc.scalar.dma_start(out=bqk_sb[:], in_=bqk)
        nc.scalar.dma_start(out=tri_sb[:], in_=tri)
        nc.scalar.dma_start(out=ident_sb[:], in_=ident)
        nc.scalar.dma_start(out=wv_sb[:], in_=wv.rearrange("(k p) c -> p k c", p=P))
        for cc in range(1, NQC):
            csl = slice(cc * QCHUNK, (cc + 1) * QCHUNK)
            nc.sync.dma_start(out=xt_sb[:, :, csl],
                              in_=xTr[:, :, csl].rearrange("k p s -> p k s"))
            if cc == 1:
                nc.scalar.dma_start(
                    out=w2_sb[:], in_=w2.rearrange("(k p) e -> p k e", p=P))

        # ---- PE p-state warmup: keep the tensor engine busy while the
        # first input DMAs land so real work starts at full clock ----
        warm = persist.tile([P, QCHUNK], BF, name="warm")
        nc.vector.memset(warm[:], 0.0)

        psA = ex.enter_context(tc.tile_pool(name="psA", bufs=2, space="PSUM"))
        psS = ex.enter_context(tc.tile_pool(name="psS", bufs=2, space="PSUM"))
        psP = ex.enter_context(tc.tile_pool(name="psP", bufs=2, space="PSUM"))
        psY = psP
        atp = ex.enter_context(tc.tile_pool(name="atp", bufs=36))
        c2p = ex.enter_context(tc.tile_pool(name="c2p", bufs=6))
        recp = ex.enter_context(tc.tile_pool(name="recp", bufs=16))
        ysbp = ex.enter_context(tc.tile_pool(name="ysbp", bufs=4))

        wps = psA.tile([P, QCHUNK], FP, tag="psa", name="wps")
        for _ in range(10):
            nc.tensor.matmul(wps[:], warm[:, 0:P], warm[:],
                             start=True, stop=True)

        def emit_qk(c, p):
            qsl = slice(c * QCHUNK, (c + 1) * QCHUNK)
            specs = ((wq_sb, qT2[p], p), (wk_sb, kT2[p], NPAIR + p))
            pss = [psA.tile([P, QCHUNK], FP, tag="psa", name="ps_qk")
                   for _ in specs]
            if c == 0:
                # startup: run both accumulation groups in lockstep with the
                # arriving x tiles so K is ready right behind Q
                for k in range(KT):
                    for s, (wsb, _, _) in enumerate(specs):
                        nc.tensor.matmul(
                            pss[s][:], wsb[:, p, k, :],
                            xt_sb[:, k, qsl],
                            start=(k == 0), stop=(k == KT - 1))
            else:
                for s, (wsb, _, _) in enumerate(specs):
                    for k in range(KT):
                        nc.tensor.matmul(
                            pss[s][:], wsb[:, p, k, :],
                            xt_sb[:, k, qsl],
                            start=(k == 0), stop=(k == KT - 1))
            for s, (_, dstT, bcol) in enumerate(specs):
                if c == 0 and s == 1:
                    # ACT is idle before the first exp: evict K there so the
                    # first score tile is not gated on two serial DVE ops
                    nc.scalar.activation(
                        dstT[:, qsl], pss[s][:],
                        mybir.ActivationFunctionType.Identity,
                        bias=bqk_sb[:, bcol:bcol + 1])
                else:
                    nc.vector.tensor_scalar_add(
                        dstT[:, qsl], pss[s][:], bqk_sb[:, bcol:bcol + 1])

        def emit_v(c):
            for t in range(4):
                tt = 4 * c + t
                ps = psA.tile([P, QCHUNK], FP, tag="psa", name="ps_v")
                for k in range(KT):
                    nc.tensor.matmul(
                        ps[:, 0:LC], xt_sb[:, k, tt * P:(tt + 1) * P],
                        wv_sb[:, k, :],
                        start=(k == 0), stop=(k == KT - 1))
                nc.vector.tensor_copy(
                    vall[:, tt, :, 0:D],
                    ps[:, 0:LC].rearrange("p (h d) -> p h d", h=LH))

        def emit_scores_off(c, h):
            """Off-diagonal score matmuls + exp for head h, chunk c: these
            need only this chunk's Q plus previous chunks' K."""
            p, half = divmod(h, 2)
            dr = slice(D * half, D * (half + 1))
            qsl = slice(c * QCHUNK, (c + 1) * QCHUNK)
            at_t = []     # per off-diagonal kt: (tile, col offset)
            for j in range(2 * c):
                ps2 = psS.tile([P, 2 * QCHUNK], FP, tag="pss", name="ps_s")
                at2 = atp.tile([P, 2 * QCHUNK], BF, tag="at", name="at")
                if True:
                    # both key tiles fully below the diagonal
                    for sub in range(2):
                        kt = 2 * j + sub
                        nc.tensor.matmul(
                            ps2[:, sub * QCHUNK:(sub + 1) * QCHUNK],
                            kT2[p][dr, kt * P:(kt + 1) * P],
                            qT2[p][dr, qsl],
                            start=True, stop=True)
                    nc.scalar.activation(at2[:], ps2[:], EXP, scale=0.125)
                    at_t.append((at2, 0))
                    at_t.append((at2, QCHUNK))
            return at_t

        def emit_scores_diag(c, h):
            """Diagonal-block score pairs for head h, chunk c (need this
            chunk's K as well)."""
            p, half = divmod(h, 2)
            dr = slice(D * half, D * (half + 1))
            at_map = {}   # diagonal kts
            for t in range(2):
                ps2 = psS.tile([P, 2 * QCHUNK], FP, tag="pss", name="ps_s")
                at2 = atp.tile([P, 2 * QCHUNK], BF, tag="at", name="at")
                # diagonal "butterfly" pair: key tiles (3,0) resp. (2,1) of
                # the diagonal block share one [128,1024] tile so the
                # trimmed regions pack with minimal dead space.  One exp
                # covers both; boundary regions (and the second pair's dead
                # gap) are zeroed below.
                # t = 0: kts (+3, +0);  1: kts (+2, +1)
                ka, kb = 4 * c + 3 - t, 4 * c + t
                da, db = (3 - t) * P, t * P
                nc.tensor.matmul(
                    ps2[:, da:QCHUNK],
                    kT2[p][dr, ka * P:(ka + 1) * P],
                    qT2[p][dr, c * QCHUNK + da:(c + 1) * QCHUNK],
                    start=True, stop=True)
                # the second half is written in FULL (not trimmed to its
                # causal delta): exp must never read unwritten PSUM -- stale
                # Inf/NaN garbage would survive the exp and turn the masked
                # boundary's 0-multiply into NaN
                nc.tensor.matmul(
                    ps2[:, QCHUNK:2 * QCHUNK],
                    kT2[p][dr, kb * P:(kb + 1) * P],
                    qT2[p][dr, c * QCHUNK:(c + 1) * QCHUNK],
                    start=True, stop=True)
                nc.scalar.activation(
                    at2[:, da:2 * QCHUNK], ps2[:, da:2 * QCHUNK],
                    EXP, scale=0.125)
                nc.vector.tensor_mul(
                    at2[:, da:da + P], at2[:, da:da + P], tri_sb[:, 0:P])
                w = db + P
                nc.vector.tensor_mul(
                    at2[:, QCHUNK:QCHUNK + w], at2[:, QCHUNK:QCHUNK + w],
                    tri_sb[:, t * P:t * P + w])
                at_map[ka] = (at2, 0)
                at_map[kb] = (at2, QCHUNK)
            return at_map

        def emit_avnorm(c, h, at_t, at_map, c2t):
            p, half = divmod(h, 2)
            if c == NQC - 1 and half == 1:
                # A-projection work is over: borrow its slots so the last
                # chunk's head chains double-buffer
                po = psA.tile([P, 4, P], FP, tag="psa", name="po_b")
            else:
                po = psP.tile([P, 4, P], FP, tag="po", name="po")
            for qt in range(4):
                gq = 4 * c + qt
                for kt in range(gq + 1):
                    att, off = at_t[kt] if kt < 4 * c else at_map[kt]
                    nc.tensor.matmul(
                        po[:, qt, 0:D + 1],
                        att[:, off + qt * P:off + (qt + 1) * P],
                        vall[:, kt, h, :],
                        start=(kt == 0), stop=(kt == gq))
            if half == 0:
                c2t[p] = c2p.tile([P, 4, P], BF, tag="c2", name="c2")
            rec = recp.tile([P, 4, 1], FP, tag="rec", name="rec")
            nc.vector.reciprocal(rec[:], po[:, :, D:D + 1])
            # broadcast the per-(token, qt) reciprocal across the 64 dims;
            # two halves so the po WAR window releases sooner
            for g in range(2):
                r2 = rec[:, 2 * g:2 * g + 2, 0:1]
                rg = bass.AP(r2.tensor, r2.offset, r2.ap[:-1] + [[0, D]])
                nc.vector.tensor_tensor(
                    c2t[p][:, 2 * g:2 * g + 2, half * D:(half + 1) * D],
                    po[:, 2 * g:2 * g + 2, 0:D], rg, mybir.AluOpType.mult)
            if half == 1:
                # both heads of the pair done: transpose to cT.  Mid-stream
                # this rides the idle DMA hardware; for the last chunk the
                # round-trip latency is on the critical tail, so use the PE
                # (stationary loads are pipelined) and evict on DVE.
                if c < NQC - 1:
                    for qt in range(4):
                        tt = 4 * c + qt
                        nc.sync.dma_start_transpose(
                            cT[p][:, tt * P:(tt + 1) * P], c2t[p][:, qt, :])
                else:
                    trp = psA.tile([P, 4, P], BF, tag="psa", name="trp")
                    for qt in range(4):
                        tt = 4 * c + qt
                        nc.tensor.transpose(
                            trp[:, qt, :], c2t[p][:, qt, :], ident_sb[:])
                        nc.vector.tensor_copy(
                            cT[p][:, tt * P:(tt + 1) * P], trp[:, qt, :])

        def emit_cproj(c):
            # c_proj runs through the wide psS slots (free once the chunk's
            # scores drain).  Mid-stream chunks evict on DVE to keep ACT on
            # exps; the last chunk evicts on the then-idle ACT, shipping
            # each half as soon as it is ready.
            last = c == NQC - 1
            for t in range(4):
                tt = 4 * c + t
                ysb = ysbp.tile([P, E], BF, tag="ysb", name="ysb")
                if last:
                    # spread the four tail tiles over four PSUM homes (two
                    # wide psS slots + the two freed po slots) so their
                    # matmul/evict/DMA chains run in parallel
                    if t < 2:
                        ps2y = psS.tile([P, 2 * QCHUNK], FP, tag="pss",
                                        name="ps_y2")
                        halves = [ps2y[:, 0:QCHUNK], ps2y[:, QCHUNK:2 * QCHUNK]]
                    else:
                        pa = psP.tile([P, QCHUNK], FP, tag="po", name="ps_ya")
                        pb = psP.tile([P, QCHUNK], FP, tag="po", name="ps_yb")
                        halves = [pa[:], pb[:]]
                    for e in range(2):
                        for ct in range(NPAIR):
                            nc.tensor.matmul(
                                halves[e],
                                cT[ct][:, tt * P:(tt + 1) * P],
                                w2_sb[:, ct, e * QCHUNK:(e + 1) * QCHUNK],
                                start=(ct == 0), stop=(ct == NPAIR - 1))
                        if e == 0:
                            nc.scalar.activation(
                                ysb[:, 0:QCHUNK], halves[0], COPY)
                        elif t == 3:
                            # terminal quarter-split: the first quarter's DMA
                            # descriptor generation overlaps the second
                            # quarter's eviction, and the kernel-ending
                            # transfer is half again as small
                            for q in range(2):
                                qs = slice(QCHUNK + q * 256,
                                           QCHUNK + (q + 1) * 256)
                                nc.vector.tensor_copy(
                                    ysb[:, qs], halves[1][:, q * 256:
                                                          (q + 1) * 256])
                                nc.sync.dma_start(
                                    out=y[tt * P:(tt + 1) * P, qs],
                                    in_=ysb[:, qs])
                        else:
                            nc.vector.tensor_copy(
                                ysb[:, QCHUNK:E], halves[1])
                        if t == 3 and e == 0:
                            # ship the ACT-evicted half immediately
                            nc.sync.dma_start(
                                out=y[tt * P:(tt + 1) * P, 0:QCHUNK],
                                in_=ysb[:, 0:QCHUNK])
                    if t != 3:
                        # the two evictions run in parallel (ACT / DVE): one
                        # full-tile DMA halves the HWDGE serialization in the
                        # kernel tail
                        nc.sync.dma_start(out=y[tt * P:(tt + 1) * P, :],
                                          in_=ysb[:])
                    continue
                for e in range(2):
                    ps = psY.tile([P, QCHUNK], FP, tag="po", name="ps_y")
                    for ct in range(NPAIR):
                        nc.tensor.matmul(
                            ps[:], cT[ct][:, tt * P:(tt + 1) * P],
                            w2_sb[:, ct, e * QCHUNK:(e + 1) * QCHUNK],
                            start=(ct == 0), stop=(ct == NPAIR - 1))
                    nc.vector.tensor_copy(
                        ysb[:, e * QCHUNK:(e + 1) * QCHUNK], ps[:])
                nc.sync.dma_start(out=y[tt * P:(tt + 1) * P, :], in_=ysb[:])

        prev = None
        for c in range(NQC):
            # each pair's scores go out as soon as that pair's Q/K are
            # projected; the PREVIOUS chunk's attn@v + normalize follow so
            # ACT always has the next chunk's exps queued before the PE
            # turns to reduction work
            c2t = [None] * NPAIR
            ats = {}
            pc, pats, pc2t = prev if prev is not None else (None, None, None)
            emit_qk(c, 0)
            ats[0] = (emit_scores_off(c, 0), emit_scores_diag(c, 0))
            if pc is not None:
                emit_avnorm(pc, 0, *pats[0], pc2t)
            ats[1] = (emit_scores_off(c, 1), emit_scores_diag(c, 1))
            if pc is not None:
                emit_avnorm(pc, 1, *pats[1], pc2t)
            emit_qk(c, 1)
            ats[2] = (emit_scores_off(c, 2), emit_scores_diag(c, 2))
            if pc is not None:
                emit_avnorm(pc, 2, *pats[2], pc2t)
            ats[3] = (emit_scores_off(c, 3), emit_scores_diag(c, 3))
            if pc is not None:
                emit_avnorm(pc, 3, *pats[3], pc2t)
            emit_v(c)
            if pc is not None:
                emit_cproj(pc)
            prev = (c, ats, c2t)
        pc, pats, pc2t = prev
        for h in range(LH):
            emit_avnorm(pc, h, *pats[h], pc2t)
        emit_cproj(pc)


_module = None


def _get_module():
    global _module
    if _module is None:
        _module = _build_module()
    return _module


def _make_tri():
    # boundary-zeroing predicates (1 = masked-out): cols 0:128 = (j < k),
    # cols 128:384 = (j < 128+k) (dead gap + boundary of a second-half tile
    # whose causal delta is one key-tile above the half boundary).
    i = np.arange(P)[:, None]
    m1 = (np.arange(P)[None, :] >= i)
    m2 = (np.arange(2 * P)[None, :] >= P + i)
    return np.concatenate([m1, m2], axis=1).astype(np.float32)


def _pack_pairs(w):
    # [E, 256] -> [128, 2*KT*128]: pair-major, k-tile-major, contiguous rows
    # so each head pair loads as a single large-descriptor DMA
    return np.ascontiguousarray(
        w.reshape(KT, P, NPAIR, P).transpose(1, 2, 0, 3).reshape(
            P, NPAIR * KT * P)).astype(ml_dtypes.bfloat16)


def kernel(hidden_states, c_attn_w, c_attn_b, c_proj_w, c_proj_b):
    hidden_states = np.asarray(hidden_states, np.float32)
    c_attn_w = np.asarray(c_attn_w, np.float32)
    c_attn_b = np.asarray(c_attn_b, np.float32)
    c_proj_w = np.asarray(c_proj_w, np.float32)
    c_proj_b = np.asarray(c_proj_b, np.float32)

    nc = _get_module()
    tri = _make_tri()
    in_maps = []
    for core in range(NCORES):
        b, g = divmod(core, HG)
        cols = slice(g * LC, (g + 1) * LC)
        # bias columns: [q pair0, q pair1, k pair0, k pair1]
        bias_cols = np.stack(
            [c_attn_b[0 * E + g * LC + p * P: 0 * E + g * LC + (p + 1) * P]
             for p in range(NPAIR)] +
            [c_attn_b[1 * E + g * LC + p * P: 1 * E + g * LC + (p + 1) * P]
             for p in range(NPAIR)], axis=1)
        in_maps.append({
            "xT": np.ascontiguousarray(hidden_states[b].T).astype(ml_dtypes.bfloat16),
            "wq": _pack_pairs(c_attn_w[:, 0 * E:1 * E][:, cols]),
            "wk": _pack_pairs(c_attn_w[:, 1 * E:2 * E][:, cols]),
            "wv": np.ascontiguousarray(c_attn_w[:, 2 * E:3 * E][:, cols]).astype(ml_dtypes.bfloat16),
            "w2": np.ascontiguousarray(c_proj_w[cols, :]).astype(ml_dtypes.bfloat16),
            "tri": tri.astype(ml_dtypes.bfloat16),
            "bqk": np.ascontiguousarray(bias_cols),
            "ident": np.eye(P, dtype=np.float32).astype(ml_dtypes.bfloat16),
        })

    global _last_in_maps
    _last_in_maps = in_maps
    res = bass_utils.run_bass_kernel_spmd(
        nc, in_maps, core_ids=list(range(NCORES)))

    # v-bias folds through attention (rows sum to 1): + bv @ Wproj + bproj
    bias_out = c_attn_b[2 * E:3 * E] @ c_proj_w + c_proj_b
    out = np.empty((B, S, E), np.float32)
    for b in range(B):
        acc = res.results[b * HG + 0]["y"].astype(np.float32).copy()
        for g in range(1, HG):
            acc += res.results[b * HG + g]["y"]
        out[b] = acc + bias_out
    return out


# revision 60
# speedup vs baseline: 1.0040x; 1.0040x over previous
"""GPT2Attention Trainium2 Bass kernel.

Problem: B=2, S=2048, E=1024, H=16 heads, D=64.
  qkv = x @ c_attn_w + c_attn_b; causal softmax attention; out @ c_proj_w + c_proj_b.

Sharding: 8 cores = 2 (batch) x 4 (head-groups of 4 heads).  Each core computes
its batch's attention for its 4 heads plus the partial c_proj contribution
(rows of c_proj_w belonging to its heads).  Host sums the 4 partials per batch
and adds the bias terms (v-bias folds through attention: attn rows sum to 1).

Device pipeline, software-pipelined over 512-query chunks so the tile
scheduler can overlap the PE-heavy projections with the ACT-heavy softmax:

  A(c): QKV projection for query chunk c.  qT/kT per head-pair [128, S]
        (partitions = 2 heads x 64 dims); v natural [tokens, 4 heads, 65]
        with a ones column that accumulates the softmax denominator.
  B(c): per head: scoresT tiles land in [128 keys, 2x512 q] PSUM pairs so a
        single ACT exp covers two key tiles; moving operands are trimmed to
        the causal boundary, and the two diagonal-block pairs pack as a
        (3,0)/(2,1) "butterfly" to minimise dead exp work.  Boundary
        regions are zeroed by an in-place bf16 multiply with a triangular
        predicate on DVE.  attn@v runs transposed -- po[128 q, 65]
        accumulates over key tiles at 65 rows/matmul instead of 512 --
        and one broadcast reciprocal-multiply normalizes a whole head.
  T(c): c2 [tok, dims] -> cT [dims, tok] via DMA-transpose mid-stream
        (idle DMA hardware); via PE-transpose for the latency-critical
        last chunk.
  C(c): partial c_proj from cT, evicted to SBUF (DVE mid-stream, the
        then-idle ACT for the last chunk) and shipped as bf16.

The emission order software-pipelines chunks: chunk c's scores/exps are
issued before chunk c-1's attn@v + normalize + c_proj, so ACT always has
exp work queued while PE turns through reductions.  A 10-matmul warmup
keeps the PE p-state ramp hot through the initial DMA window.
"""

from contextlib import ExitStack

import numpy as np
import ml_dtypes

import bass_rust
import concourse.bass as bass
import concourse.tile as tile
from concourse import mybir
from concourse import bass_utils


def _patched_drain_and_barrier(self, tick_clock, wait_clock):
    # The stock walrus in this container rejects instructions carrying more
    # than one sync wait ("Too many sync wait commands" on the kernel-tail
    # Drain).  Spread the final waits across single-wait NOPs instead.
    nc = self.nc
    probe = nc.sync.nop()
    wait_clock.add_sem_waits(
        probe.ins, bass_rust.ScopedClock({None: tick_clock.global_clock}))
    si = probe.ins.sync_info
    waits = list(si.on_wait) if si is not None else []
    if len(waits) > 1:
        probe.ins.sync_info = mybir.SyncInfo(
            on_wait=waits[:1], on_update=list(si.on_update))
        for w in waits[1:]:
            n = nc.sync.nop()
            n.ins.sync_info = mybir.SyncInfo(on_wait=[w], on_update=[])
    nc.sync.drain()
    nc.all_engine_barrier()
    assert self.sems is not None
    popped = nc._tile_sem_poison_stack.pop()
    assert popped is self._sem_poison
    nc.clear_and_free_semaphores(list(self.sems.allocated().values()))
    nc.all_engine_barrier()


tile.TileContext._drain_and_barrier = _patched_drain_and_barrier

_split_ctr = [0]


def _split_sync_waits(nc):
    """Stock walrus allows one sync wait per instruction; hoist extras onto
    single-wait NOPs inserted just before, on the same (in-order) engine."""
    for fn in nc.m.functions:
        for bb in fn.blocks:
            insts = bb.instructions
            out = []
            changed = False
            for inst in insts:
                si = getattr(inst, "sync_info", None)
                waits = list(si.on_wait) if si is not None else []
                if len(waits) > 1:
                    changed = True
                    for w in waits[:-1]:
                        _split_ctr[0] += 1
                        nop = bass_rust.InstNoOp(
                            name=f"I-syncsplit-{_split_ctr[0]}",
                            engine=inst.engine)
                        nop.sync_info = mybir.SyncInfo(on_wait=[w], on_update=[])
                        out.append(nop)
                    inst.sync_info = mybir.SyncInfo(
                        on_wait=[waits[-1]], on_update=list(si.on_update))
                out.append(inst)
            if changed:
                bb.instructions = out

B, S, E, H, D = 2, 2048, 1024, 16, 64
NCORES = 8
HG = 4                # head-group cores per batch
LH = H // HG          # 4 local heads per core
LC = LH * D           # 256 local c_proj rows
NPAIR = LH // 2       # 2 head-pairs per core
P = 128
KT = E // P           # 8 contraction tiles for the projections
QCHUNK = 512
NQC = S // QCHUNK     # 4 query chunks
NKT = S // P          # 16 key tiles
NTT = S // P          # 16 token tiles

FP = mybir.dt.float32
BF = mybir.dt.bfloat16
EXP = mybir.ActivationFunctionType.Exp
COPY = mybir.ActivationFunctionType.Copy


def _build_module():
    nc = bass.Bass("TRN2", target_bir_lowering=False, debug=False,
                   num_devices=NCORES)
    xT = nc.dram_tensor("xT", [E, S], BF, kind="ExternalInput").ap()
    wq = nc.dram_tensor("wq", [P, NPAIR * KT * P], BF, kind="ExternalInput").ap()
    wk = nc.dram_tensor("wk", [P, NPAIR * KT * P], BF, kind="ExternalInput").ap()
    wv = nc.dram_tensor("wv", [E, LC], BF, kind="ExternalInput").ap()
    w2 = nc.dram_tensor("w2", [LC, E], BF, kind="ExternalInput").ap()
    tri = nc.dram_tensor("tri", [P, 3 * P], BF, kind="ExternalInput").ap()
    bqk = nc.dram_tensor("bqk", [P, 2 * NPAIR], FP, kind="ExternalInput").ap()
    ident = nc.dram_tensor("ident", [P, P], BF, kind="ExternalInput").ap()
    y = nc.dram_tensor("y", [S, E], BF, kind="ExternalOutput").ap()

    with tile.TileContext(nc) as tc:
        _body(tc, xT, wq, wk, wv, w2, tri, bqk, ident, y)
    _split_sync_waits(nc)
    return nc


def _body(tc, xT, wq, wk, wv, w2, tri, bqk, ident, y):
    nc = tc.nc
    ex = ExitStack()
    with ex:
        persist = ex.enter_context(tc.tile_pool(name="persist", bufs=1))

        # ---- persistent tiles ----
        qT2 = [persist.tile([P, S], BF, name=f"qT2_{p}") for p in range(NPAIR)]
        kT2 = [persist.tile([P, S], BF, name=f"kT2_{p}") for p in range(NPAIR)]
        # v natural layout: [token-partitions, ttile, head, dim(+denom col)]
        vall = persist.tile([P, NTT, LH, D + 1], BF, name="vall")
        cT = [persist.tile([P, S], BF, name=f"cT_{p}") for p in range(NPAIR)]
        xt_sb = persist.tile([P, KT, S], BF, name="xt_sb")
        wq_sb = persist.tile([P, NPAIR, KT, P], BF, name="wq_sb")
        wk_sb = persist.tile([P, NPAIR, KT, P], BF, name="wk_sb")
        wv_sb = persist.tile([P, KT, LC], BF, name="wv_sb")
        w2_sb = persist.tile([P, 2, E], BF, name="w2_sb")
        # masking predicates for the causal boundary: cols 0:128 = (j < k),
        # 128:384 = (j < 128+k)
        tri_sb = persist.tile([P, 3 * P], BF, name="tri_sb")
        bqk_sb = persist.tile([P, 2 * NPAIR], FP, name="bqk_sb")
        ident_sb = persist.tile([P, P], BF, name="ident_sb")

        nc.vector.memset(vall[:, :, :, D:D + 1], 1.0)

        # ---- input DMAs (ordered so chunk-0 work can start early) ----
        xTr = xT.rearrange("(k p) s -> k p s", p=P)
        KP = KT * P
        nc.sync.dma_start(out=wq_sb[:, 0], in_=wq[:, 0:KP].rearrange(
            "p (k j) -> p k j", k=KT))
        nc.sync.dma_start(out=wk_sb[:, 0], in_=wk[:, 0:KP].rearrange(
            "p (k j) -> p k j", k=KT))
        nc.sync.dma_start(out=xt_sb[:, 0:4, 0:QCHUNK],
                          in_=xTr[0:4, :, 0:QCHUNK].rearrange("k p s -> p k s"))
        nc.sync.dma_start(out=xt_sb[:, 4:8, 0:QCHUNK],
                          in_=xTr[4:8, :, 0:QCHUNK].rearrange("k p s -> p k s"))
        nc.sync.dma_start(out=wq_sb[:, 1], in_=wq[:, KP:2 * KP].rearrange(
            "p (k j) -> p k j", k=KT))
        nc.sync.dma_start(out=wk_sb[:, 1], in_=wk[:, KP:2 * KP].rearrange(
            "p (k j) -> p k j", k=KT))
        nc.scalar.dma_start(out=bqk_sb[:], in_=bqk)
        nc.scalar.dma_start(out=tri_sb[:], in_=tri)
        nc.scalar.dma_start(out=ident_sb[:], in_=ident)
        nc.scalar.dma_start(out=wv_sb[:], in_=wv.rearrange("(k p) c -> p k c", p=P))
        for cc in range(1, NQC):
            csl = slice(cc * QCHUNK, (cc + 1) * QCHUNK)
            nc.sync.dma_start(out=xt_sb[:, :, csl],
                              in_=xTr[:, :, csl].rearrange("k p s -> p k s"))
            if cc == 1:
                nc.scalar.dma_start(
                    out=w2_sb[:], in_=w2.rearrange("(k p) e -> p k e", p=P))

        # ---- PE p-state warmup: keep the tensor engine busy while the
        # first input DMAs land so real work starts at full clock ----
        warm = persist.tile([P, QCHUNK], BF, name="warm")
        nc.vector.memset(warm[:], 0.0)

        psA = ex.enter_context(tc.tile_pool(name="psA", bufs=2, space="PSUM"))
        psS = ex.enter_context(tc.tile_pool(name="psS", bufs=2, space="PSUM"))
        psP = ex.enter_context(tc.tile_pool(name="psP", bufs=2, space="PSUM"))
        psY = psP
        atp = ex.enter_context(tc.tile_pool(name="atp", bufs=36))
        c2p = ex.enter_context(tc.tile_pool(name="c2p", bufs=6))
        recp = ex.enter_context(tc.tile_pool(name="recp", bufs=16))
        ysbp = ex.enter_context(tc.tile_pool(name="ysbp", bufs=4))

        wps = psA.tile([P, QCHUNK], FP, tag="psa", name="wps")
        for _ in range(10):
            nc.tensor.matmul(wps[:], warm[:, 0:P], warm[:],
                             start=True, stop=True)

        def emit_qk(c, p):
            qsl = slice(c * QCHUNK, (c + 1) * QCHUNK)
            specs = ((wq_sb, qT2[p], p), (wk_sb, kT2[p], NPAIR + p))
            pss = [psA.tile([P, QCHUNK], FP, tag="psa", name="ps_qk")
                   for _ in specs]
            if c == 0:
                # startup: run both accumulation groups in lockstep with the
                # arriving x tiles so K is ready right behind Q
                for k in range(KT):
                    for s, (wsb, _, _) in enumerate(specs):
                        nc.tensor.matmul(
                            pss[s][:], wsb[:, p, k, :],
                            xt_sb[:, k, qsl],
                            start=(k == 0), stop=(k == KT - 1))
            else:
                for s, (wsb, _, _) in enumerate(specs):
                    for k in range(KT):
                        nc.tensor.matmul(
                            pss[s][:], wsb[:, p, k, :],
                            xt_sb[:, k, qsl],
                            start=(k == 0), stop=(k == KT - 1))
            for s, (_, dstT, bcol) in enumerate(specs):
                if c == 0 and s == 1:
                    # ACT is idle before the first exp: evict K there so the
                    # first score tile is not gated on two serial DVE ops
                    nc.scalar.activation(
                        dstT[:, qsl], pss[s][:],
                        mybir.ActivationFunctionType.Identity,
                        bias=bqk_sb[:, bcol:bcol + 1])
                else:
                    nc.vector.tensor_scalar_add(
                        dstT[:, qsl], pss[s][:], bqk_sb[:, bcol:bcol + 1])

        def emit_v(c):
            for t in range(4):
                tt = 4 * c + t
                ps = psA.tile([P, QCHUNK], FP, tag="psa", name="ps_v")
                for k in range(KT):
                    nc.tensor.matmul(
                        ps[:, 0:LC], xt_sb[:, k, tt * P:(tt + 1) * P],
                        wv_sb[:, k, :],
                        start=(k == 0), stop=(k == KT - 1))
                nc.vector.tensor_copy(
                    vall[:, tt, :, 0:D],
                    ps[:, 0:LC].rearrange("p (h d) -> p h d", h=LH))

        def emit_scores_off(c, h):
            """Off-diagonal score matmuls + exp for head h, chunk c: these
            need only this chunk's Q plus previous chunks' K."""
            p, half = divmod(h, 2)
            dr = slice(D * half, D * (half + 1))
            qsl = slice(c * QCHUNK, (c + 1) * QCHUNK)
            at_t = []     # per off-diagonal kt: (tile, col offset)
            for j in range(2 * c):
                ps2 = psS.tile([P, 2 * QCHUNK], FP, tag="pss", name="ps_s")
                at2 = atp.tile([P, 2 * QCHUNK], BF, tag="at", name="at")
                if True:
                    # both key tiles fully below the diagonal
                    for sub in range(2):
                        kt = 2 * j + sub
                        nc.tensor.matmul(
                            ps2[:, sub * QCHUNK:(sub + 1) * QCHUNK],
                            kT2[p][dr, kt * P:(kt + 1) * P],
                            qT2[p][dr, qsl],
                            start=True, stop=True)
                    nc.scalar.activation(at2[:], ps2[:], EXP, scale=0.125)
                    at_t.append((at2, 0))
                    at_t.append((at2, QCHUNK))
            return at_t

        def emit_scores_diag(c, h):
            """Diagonal-block score pairs for head h, chunk c (need this
            chunk's K as well)."""
            p, half = divmod(h, 2)
            dr = slice(D * half, D * (half + 1))
            at_map = {}   # diagonal kts
            for t in range(2):
                ps2 = psS.tile([P, 2 * QCHUNK], FP, tag="pss", name="ps_s")
                at2 = atp.tile([P, 2 * QCHUNK], BF, tag="at", name="at")
                # diagonal "butterfly" pair: key tiles (3,0) resp. (2,1) of
                # the diagonal block share one [128,1024] tile so the
                # trimmed regions pack with minimal dead space.  One exp
                # covers both; boundary regions (and the second pair's dead
                # gap) are zeroed below.
                # t = 0: kts (+3, +0);  1: kts (+2, +1)
                ka, kb = 4 * c + 3 - t, 4 * c + t
                da, db = (3 - t) * P, t * P
                nc.tensor.matmul(
                    ps2[:, da:QCHUNK],
                    kT2[p][dr, ka * P:(ka + 1) * P],
                    qT2[p][dr, c * QCHUNK + da:(c + 1) * QCHUNK],
                    start=True, stop=True)
                # the second half is written in FULL (not trimmed to its
                # causal delta): exp must never read unwritten PSUM -- stale
                # Inf/NaN garbage would survive the exp and turn the masked
                # boundary's 0-multiply into NaN
                nc.tensor.matmul(
                    ps2[:, QCHUNK:2 * QCHUNK],
                    kT2[p][dr, kb * P:(kb + 1) * P],
                    qT2[p][dr, c * QCHUNK:(c + 1) * QCHUNK],
                    start=True, stop=True)
                nc.scalar.activation(
                    at2[:, da:2 * QCHUNK], ps2[:, da:2 * QCHUNK],
                    EXP, scale=0.125)
                nc.vector.tensor_mul(
                    at2[:, da:da + P], at2[:, da:da + P], tri_sb[:, 0:P])
                w = db + P
                nc.vector.tensor_mul(
                    at2[:, QCHUNK:QCHUNK + w], at2[:, QCHUNK:QCHUNK + w],
                    tri_sb[:, t * P:t * P + w])
                at_map[ka] = (at2, 0)
                at_map[kb] = (at2, QCHUNK)
            return at_map

        def emit_avnorm(c, h, at_t, at_map, c2t):
            p, half = divmod(h, 2)
            if c == NQC - 1 and half == 1:
                # A-projection work is over: borrow its slots so the last
                # chunk's head chains double-buffer
                po = psA.tile([P, 4, P], FP, tag="psa", name="po_b")
            else:
                po = psP.tile([P, 4, P], FP, tag="po", name="po")
            for qt in range(4):
                gq = 4 * c + qt
                for kt in range(gq + 1):
                    att, off = at_t[kt] if kt < 4 * c else at_map[kt]
                    nc.tensor.matmul(
                        po[:, qt, 0:D + 1],
                        att[:, off + qt * P:off + (qt + 1) * P],
                        vall[:, kt, h, :],
                        start=(kt == 0), stop=(kt == gq))
            if half == 0:
                c2t[p] = c2p.tile([P, 4, P], BF, tag="c2", name="c2")
            rec = recp.tile([P, 4, 1], FP, tag="rec", name="rec")
            nc.vector.reciprocal(rec[:], po[:, :, D:D + 1])
            # broadcast the per-(token, qt) reciprocal across the 64 dims;
            # two halves so the po WAR window releases sooner
            for g in range(2):
                r2 = rec[:, 2 * g:2 * g + 2, 0:1]
                rg = bass.AP(r2.tensor, r2.offset, r2.ap[:-1] + [[0, D]])
                nc.vector.tensor_tensor(
                    c2t[p][:, 2 * g:2 * g + 2, half * D:(half + 1) * D],
                    po[:, 2 * g:2 * g + 2, 0:D], rg, mybir.AluOpType.mult)
            if half == 1:
                # both heads of the pair done: transpose to cT.  Mid-stream
                # this rides the idle DMA hardware; for the last chunk the
                # round-trip latency is on the critical tail, so use the PE
                # (stationary loads are pipelined) and evict on DVE.
                if c < NQC - 1:
                    for qt in range(4):
                        tt = 4 * c + qt
                        nc.sync.dma_start_transpose(
                            cT[p][:, tt * P:(tt + 1) * P], c2t[p][:, qt, :])
                else:
                    trp = psA.tile([P, 4, P], BF, tag="psa", name="trp")
                    for qt in range(4):
                        tt = 4 * c + qt
                        nc.tensor.transpose(
                            trp[:, qt, :], c2t[p][:, qt, :], ident_sb[:])
                        nc.vector.tensor_copy(
                            cT[p][:, tt * P:(tt + 1) * P], trp[:, qt, :])

        def emit_cproj(c):
            # c_proj runs through the wide psS slots (free once the chunk's
            # scores drain).  Mid-stream chunks evict on DVE to keep ACT on
            # exps; the last chunk evicts on the then-idle ACT, shipping
            # each half as soon as it is ready.
            last = c == NQC - 1
            for t in range(4):
                tt = 4 * c + t
                ysb = ysbp.tile([P, E], BF, tag="ysb", name="ysb")
                if last:
                    # spread the four tail tiles over four PSUM homes (two
                    # wide psS slots + the two freed po slots) so their
                    # matmul/evict/DMA chains run in parallel
                    if t < 2:
                        ps2y = psS.tile([P, 2 * QCHUNK], FP, tag="pss",
                                        name="ps_y2")
                        halves = [ps2y[:, 0:QCHUNK], ps2y[:, QCHUNK:2 * QCHUNK]]
                    else:
                        pa = psP.tile([P, QCHUNK], FP, tag="po", name="ps_ya")
                        pb = psP.tile([P, QCHUNK], FP, tag="po", name="ps_yb")
                        halves = [pa[:], pb[:]]
                    for e in range(2):
                        for ct in range(NPAIR):
                            nc.tensor.matmul(
                                halves[e],
                                cT[ct][:, tt * P:(tt + 1) * P],
                                w2_sb[:, ct, e * QCHUNK:(e + 1) * QCHUNK],
                                start=(ct == 0), stop=(ct == NPAIR - 1))
                        if e == 0:
                            nc.scalar.activation(
                                ysb[:, 0:QCHUNK], halves[0], COPY)
                        else:
                            nc.vector.tensor_copy(
                                ysb[:, QCHUNK:E], halves[1])
                        if t == 3:
                            # terminal tile: ship each half as soon as its
                            # eviction lands so the kernel-ending DMA is half
                            # the size
                            nc.sync.dma_start(
                                out=y[tt * P:(tt + 1) * P,
                                      e * QCHUNK:(e + 1) * QCHUNK],
                                in_=ysb[:, e * QCHUNK:(e + 1) * QCHUNK])
                    if t != 3:
                        # the two evictions run in parallel (ACT / DVE): one
                        # full-tile DMA halves the HWDGE serialization in the
                        # kernel tail
                        nc.sync.dma_start(out=y[tt * P:(tt + 1) * P, :],
                                          in_=ysb[:])
                    continue
                for e in range(2):
                    ps = psY.tile([P, QCHUNK], FP, tag="po", name="ps_y")
                    for ct in range(NPAIR):
                        nc.tensor.matmul(
                            ps[:], cT[ct][:, tt * P:(tt + 1) * P],
                            w2_sb[:, ct, e * QCHUNK:(e + 1) * QCHUNK],
                            start=(ct == 0), stop=(ct == NPAIR - 1))
                    nc.vector.tensor_copy(
                        ysb[:, e * QCHUNK:(e + 1) * QCHUNK], ps[:])
                nc.sync.dma_start(out=y[tt * P:(tt + 1) * P, :], in_=ysb[:])

        prev = None
        for c in range(NQC):
            # each pair's scores go out as soon as that pair's Q/K are
            # projected; the PREVIOUS chunk's attn@v + normalize follow so
            # ACT always has the next chunk's exps queued before the PE
            # turns to reduction work
            c2t = [None] * NPAIR
            ats = {}
            pc, pats, pc2t = prev if prev is not None else (None, None, None)
            emit_qk(c, 0)
            ats[0] = (emit_scores_off(c, 0), emit_scores_diag(c, 0))
            if pc is not None:
                emit_avnorm(pc, 0, *pats[0], pc2t)
            ats[1] = (emit_scores_off(c, 1), emit_scores_diag(c, 1))
            if pc is not None:
                emit_avnorm(pc, 1, *pats[1], pc2t)
            emit_qk(c, 1)
            ats[2] = (emit_scores_off(c, 2), emit_scores_diag(c, 2))
            if pc is not None:
                emit_avnorm(pc, 2, *pats[2], pc2t)
            ats[3] = (emit_scores_off(c, 3), emit_scores_diag(c, 3))
            if pc is not None:
                emit_avnorm(pc, 3, *pats[3], pc2t)
            emit_v(c)
            if pc is not None:
                emit_cproj(pc)
            prev = (c, ats, c2t)
        pc, pats, pc2t = prev
        for h in range(LH):
            emit_avnorm(pc, h, *pats[h], pc2t)
        emit_cproj(pc)


_module = None


def _get_module():
    global _module
    if _module is None:
        _module = _build_module()
    return _module


def _make_tri():
    # boundary-zeroing predicates (1 = masked-out): cols 0:128 = (j < k),
    # cols 128:384 = (j < 128+k) (dead gap + boundary of a second-half tile
    # whose causal delta is one key-tile above the half boundary).
    i = np.arange(P)[:, None]
    m1 = (np.arange(P)[None, :] >= i)
    m2 = (np.arange(2 * P)[None, :] >= P + i)
    return np.concatenate([m1, m2], axis=1).astype(np.float32)


def _pack_pairs(w):
    # [E, 256] -> [128, 2*KT*128]: pair-major, k-tile-major, contiguous rows
    # so each head pair loads as a single large-descriptor DMA
    return np.ascontiguousarray(
        w.reshape(KT, P, NPAIR, P).transpose(1, 2, 0, 3).reshape(
            P, NPAIR * KT * P)).astype(ml_dtypes.bfloat16)


def kernel(hidden_states, c_attn_w, c_attn_b, c_proj_w, c_proj_b):
    hidden_states = np.asarray(hidden_states, np.float32)
    c_attn_w = np.asarray(c_attn_w, np.float32)
    c_attn_b = np.asarray(c_attn_b, np.float32)
    c_proj_w = np.asarray(c_proj_w, np.float32)
    c_proj_b = np.asarray(c_proj_b, np.float32)

    nc = _get_module()
    tri = _make_tri()
    in_maps = []
    for core in range(NCORES):
        b, g = divmod(core, HG)
        cols = slice(g * LC, (g + 1) * LC)
        # bias columns: [q pair0, q pair1, k pair0, k pair1]
        bias_cols = np.stack(
            [c_attn_b[0 * E + g * LC + p * P: 0 * E + g * LC + (p + 1) * P]
             for p in range(NPAIR)] +
            [c_attn_b[1 * E + g * LC + p * P: 1 * E + g * LC + (p + 1) * P]
             for p in range(NPAIR)], axis=1)
        in_maps.append({
            "xT": np.ascontiguousarray(hidden_states[b].T).astype(ml_dtypes.bfloat16),
            "wq": _pack_pairs(c_attn_w[:, 0 * E:1 * E][:, cols]),
            "wk": _pack_pairs(c_attn_w[:, 1 * E:2 * E][:, cols]),
            "wv": np.ascontiguousarray(c_attn_w[:, 2 * E:3 * E][:, cols]).astype(ml_dtypes.bfloat16),
            "w2": np.ascontiguousarray(c_proj_w[cols, :]).astype(ml_dtypes.bfloat16),
            "tri": tri.astype(ml_dtypes.bfloat16),
            "bqk": np.ascontiguousarray(bias_cols),
            "ident": np.eye(P, dtype=np.float32).astype(ml_dtypes.bfloat16),
        })

    global _last_in_maps
    _last_in_maps = in_maps
    res = bass_utils.run_bass_kernel_spmd(
        nc, in_maps, core_ids=list(range(NCORES)))

    # v-bias folds through attention (rows sum to 1): + bv @ Wproj + bproj
    bias_out = c_attn_b[2 * E:3 * E] @ c_proj_w + c_proj_b
    out = np.empty((B, S, E), np.float32)
    for b in range(B):
        acc = res.results[b * HG + 0]["y"].astype(np.float32).copy()
        for g in range(1, HG):
            acc += res.results[b * HG + g]["y"]
        out[b] = acc + bias_out
    return out
